# revision 35
# baseline (speedup 1.0000x reference)
"""MoE (top-2 routed SwiGLU) kernel for 8 Trainium2 NeuronCores.

Strategy (expert-parallel, host-routed dispatch):
  * Host: router matmul x@Wg.T (+bg), top-k + softmax weights, sort tokens
    by expert, pad each expert's token list to a shared capacity C.
  * Device (SPMD over 8 cores, core e owns expert e):
        OUT_e[D, C] = W2_e @ (silu(W1_e @ X_e) * (W3_e @ X_e))
    All matmul operands are bf16 (fp32 PSUM accumulate): bf16 gets the
    FWL fast weight-load path so LDWEIGHTS (~107ns) hides fully behind
    the N-column fill, and weight/input DMA bytes halve vs fp32.
    A burst of dummy matmuls at kernel start revs the HAM clock to
    full rate (2.4GHz) while the startup DMAs stream.
    Weights stream through SBUF once, chunked over the DFF axis; the
    output accumulates in SBUF across DFF chunks.
  * Host: y[tok] += w_tok_e * OUT_e[:, pos].T over the k experts per token.

Only top-k expert work is computed (4x less than the dense reference).
"""

import math
import sys

import numpy as np

for _p in ("/opt/trn_rl_repo", "/opt/pypackages"):
    if _p not in sys.path:
        sys.path.append(_p)

import ml_dtypes  # noqa: E402

import concourse.bass as bass  # noqa: E402
import concourse.tile as tile  # noqa: E402
from concourse import bacc, bass_utils, mybir  # noqa: E402

F32 = mybir.dt.float32
BF16 = mybir.dt.bfloat16
AF = mybir.ActivationFunctionType
NP_BF16 = ml_dtypes.bfloat16

D, DFF, E = 1024, 4096, 8
NCORES = 8
P = 128
KC = D // P            # 8 contraction chunks for the first matmuls
DFFC = 512             # dff columns per weight-stream chunk
NF = DFF // DFFC       # 8 weight-stream iterations
MC2 = DFFC // P        # 4 contraction chunks for the second matmul
C_CAP = 1280           # max padded tokens per expert per device pass
N_WARM = 28            # dummy matmuls to rev the HAM clock at start

LAST_RESULTS = []      # BassKernelResults per device pass (for test harness)
_NC_CACHE = {}


def _install_ntff_hook():
    """Best-effort: register the axon NTFF profile hook so that
    BASS_TRACE=1 yields exec_time_ns even in a bare environment."""
    try:
        import types
        if "antenv.axon_hooks" not in sys.modules:
            mod = types.ModuleType("antenv.axon_hooks")
            holder = {}
            mod.set_axon_ntff_profile_hook = lambda h: holder.__setitem__("h", h)
            mod.get_axon_ntff_profile_hook = lambda: holder.get("h")
            sys.modules["antenv.axon_hooks"] = mod
            import antenv
            antenv.axon_hooks = mod
        mod = sys.modules["antenv.axon_hooks"]
        if mod.get_axon_ntff_profile_hook() is None:
            from trn_agent_boot.trn_boot import _ntff_profile_via_ctypes
            hook = _ntff_profile_via_ctypes("/opt/axon/libaxon_pjrt.so")
            if hook is not None:
                mod.set_axon_ntff_profile_hook(hook)
    except Exception:
        pass


_install_ntff_hook()


def _token_blocks(C):
    """Split C into even-sized blocks of <=512 columns (PSUM bank limit).
    A 256-col first block starts the PE on a small DMA prefix while
    keeping x rows >=512B (full DMA line rate); its ~3.4us of chunk-0
    work covers the second x block's wire time."""
    if C <= 512:
        return [(0, C)]
    first = 256
    rem = C - first
    n = max(1, math.ceil(rem / 500))
    base = ((rem // n) // 2) * 2
    extra = (rem - n * base) // 2         # blocks that get +2
    sizes = [first] + [base + 2] * extra + [base] * (n - extra)
    blocks, t0 = [], 0
    for sz in sizes:
        blocks.append((t0, sz))
        t0 += sz
    assert t0 == C and all(sz <= 512 for sz in sizes), (C, sizes)
    return blocks


def _build(C):
    """Compile the per-core expert-FFN program for capacity C."""
    if C in _NC_CACHE:
        return _NC_CACHE[C]
    nc = bacc.Bacc(
        "TRN2", target_bir_lowering=False, debug=False, num_devices=NCORES
    )
    x_d = nc.dram_tensor("xt", [D, C], BF16, kind="ExternalInput")
    w1_d = nc.dram_tensor("w1", [D, DFF], BF16, kind="ExternalInput")
    w3_d = nc.dram_tensor("w3", [D, DFF], BF16, kind="ExternalInput")
    w2_d = nc.dram_tensor("w2", [DFF, D], BF16, kind="ExternalInput")
    o_d = nc.dram_tensor("out", [D, C], F32, kind="ExternalOutput")

    xr = x_d.ap().rearrange("(kc p) c -> p kc c", p=P)
    w1r = w1_d.ap().rearrange("(kc p) f -> p kc f", p=P)
    w3r = w3_d.ap().rearrange("(kc p) f -> p kc f", p=P)
    w2r = w2_d.ap().rearrange("(kc p) d -> p kc d", p=P)
    orr = o_d.ap().rearrange("(mo p) c -> p mo c", p=P)

    tblocks = _token_blocks(C)
    # DFF chunk schedule: the first chunk is 256 wide so the startup
    # DMA prefix (first x block + 2x 0.5MB of weights, all >=512B rows
    # for full DMA line rate) is small and the PE starts early; steady
    # state streams 512-wide chunks.
    chunks = [(0, 256), (256, 256)]
    off = 512
    while off < DFF:
        chunks.append((off, 512))
        off += 512
    n_chunks = len(chunks)

    with tile.TileContext(nc) as tc:
        with (
            tc.tile_pool(name="res", bufs=1) as res,
            tc.tile_pool(name="w13", bufs=2) as w13,
            tc.tile_pool(name="w2p", bufs=3) as w2p,
            tc.tile_pool(name="hp", bufs=3) as hp,
            tc.tile_pool(name="sp", bufs=3) as sp,
            tc.tile_pool(name="ps13", bufs=2, space="PSUM") as ps13,
            tc.tile_pool(name="pso", bufs=3, space="PSUM") as pso,
            tc.tile_pool(name="psw", bufs=1, space="PSUM") as psw,
        ):
            xt = res.tile([P, KC, C], BF16, tag="xt")
            acc = res.tile([P, KC, C], F32, tag="acc")

            # HAM pre-warm: a burst of tiny matmuls revs the PE clock to
            # full rate while the startup DMAs stream in parallel.
            dwm = res.tile([P, P], BF16, tag="dwm")
            nc.vector.memset(dwm[:, :], 0.0)
            pd = psw.tile([P, P], F32, tag="pd")
            for _ in range(N_WARM):
                nc.tensor.matmul(pd[:, :], dwm[:, :], dwm[:, :],
                                 start=True, stop=True)

            # dma_start costs 0.6-3us of SERIAL issue time on the
            # issuing engine (scaling with descriptor rows), and issue
            # from multiple engines contends at the shared DGE, so:
            # whole-tile DMAs, one issuing engine, strict need-order.
            #
            # The DMA ring interleaves packets of queued transfers, so a
            # startup-critical transfer is slowed by later ones sharing
            # the ring. gate_src forces ordering: a 2-column scalar copy
            # from the previous critical tile into the next DMA's
            # destination region makes that DMA wait (WAW) until the
            # previous transfer has fully landed.
            def gate(dst_ap, src_ap):
                nc.scalar.activation(dst_ap, src_ap, AF.Copy)

            def load_x(t0, nt, gate_src=None, eng=None):
                if gate_src is not None:
                    gate(xt[:, 0, t0:t0 + 2], gate_src)
                (eng or nc.sync).dma_start(
                    xt[:, :, t0:t0 + nt], xr[:, :, t0:t0 + nt]
                )

            def load_w13(fs, dffc, eng=None):
                w1t = w13.tile([P, KC, DFFC], BF16, tag="w1")
                w3t = w13.tile([P, KC, DFFC], BF16, tag="w3")
                e = eng or nc.sync
                e.dma_start(w1t[:, :, :dffc], w1r[:, :, fs:fs + dffc])
                e.dma_start(w3t[:, :, :dffc], w3r[:, :, fs:fs + dffc])
                return w1t, w3t

            def load_w2(fs, dffc, eng=None):
                w2t = w2p.tile([P, MC2, D], BF16, tag="w2")
                mc2 = dffc // P
                (eng or nc.sync).dma_start(
                    w2t[:, :mc2, :], w2r[:, fs // P:fs // P + mc2, :]
                )
                return w2t

            # Startup: the critical prefix (x block 0, chunk-0 W1/W3,
            # then x block 1 gated behind them) owns the sync HWDGE ring
            # exclusively; everything else needed early rides the
            # independent gpsimd SWDGE ring in parallel.
            load_x(*tblocks[0])
            w13_of = {0: load_w13(0, chunks[0][1])}
            if len(tblocks) > 1:
                load_x(*tblocks[1], gate_src=w13_of[0][1][:, 0, 0:2])
            w2_of = {0: load_w2(0, chunks[0][1], eng=nc.gpsimd)}
            for tb in tblocks[2:]:
                load_x(*tb, eng=nc.gpsimd)

            def emit_stage_ph(fc, t0, nt):
                """First matmul pair + silu*mul for one (chunk, tblock)."""
                dffc = chunks[fc][1]
                h = hp.tile([P, MC2, 512], BF16, tag="h")
                for m in range(dffc // P):
                    (w1s, w3s), cl = w13_of[fc], m * P
                    ph1 = ps13.tile([P, 512], F32, tag="ph1")
                    ph3 = ps13.tile([P, 512], F32, tag="ph3")
                    for k in range(KC):
                        nc.tensor.matmul(
                            ph1[:, :nt],
                            w1s[:, k, cl:cl + P],
                            xt[:, k, t0:t0 + nt],
                            start=(k == 0),
                            stop=(k == KC - 1),
                        )
                    for k in range(KC):
                        nc.tensor.matmul(
                            ph3[:, :nt],
                            w3s[:, k, cl:cl + P],
                            xt[:, k, t0:t0 + nt],
                            start=(k == 0),
                            stop=(k == KC - 1),
                        )
                    s = sp.tile([P, 512], BF16, tag="s")
                    nc.scalar.activation(s[:, :nt], ph1[:, :nt], AF.Silu)
                    nc.vector.tensor_mul(h[:, m, :nt], s[:, :nt], ph3[:, :nt])
                return h

            def emit_stage_w2(fc, t0, nt, h, final):
                """Second matmul + accumulate for one (chunk, tblock)."""
                fs, dffc = chunks[fc]
                mc2 = dffc // P
                w2t = w2_of[fc]
                for mo in range(KC):
                    po = pso.tile([P, 512], F32, tag="po")
                    for j in range(mc2):
                        nc.tensor.matmul(
                            po[:, :nt],
                            w2t[:, j, mo * P:(mo + 1) * P],
                            h[:, j, :nt],
                            start=(j == 0),
                            stop=(j == mc2 - 1),
                        )
                    if fc == 0:
                        nc.scalar.activation(
                            acc[:, mo, t0:t0 + nt], po[:, :nt], AF.Copy
                        )
                    else:
                        nc.vector.tensor_add(
                            acc[:, mo, t0:t0 + nt],
                            acc[:, mo, t0:t0 + nt],
                            po[:, :nt],
                        )
                    # Stream the finished output out while later stages
                    # still compute. The final stage is the small token
                    # block, drained in two halves so the second half's
                    # issue+transfer is all that trails the last matmul.
                    if fc == n_chunks - 1:
                        if not final:
                            nc.sync.dma_start(
                                orr[:, mo, t0:t0 + nt],
                                acc[:, mo, t0:t0 + nt],
                            )
                        elif mo in (KC // 2 - 1, KC - 1):
                            lo = 0 if mo == KC // 2 - 1 else KC // 2
                            nc.sync.dma_start(
                                orr[:, lo:mo + 1, t0:t0 + nt],
                                acc[:, lo:mo + 1, t0:t0 + nt],
                            )

            # Software pipeline: the W2 stage lags one (chunk, tblock)
            # stage behind the ph stage, so the PE never waits on the
            # scalar/vector silu*mul chain at stage boundaries.
            prev = None
            for fc in range(n_chunks):
                if fc >= 1:
                    fs, dffc = chunks[fc]
                    e = nc.gpsimd if fc == 1 else None
                    w13_of[fc] = load_w13(fs, dffc, eng=e)
                    w2_of[fc] = load_w2(fs, dffc, eng=e)
                # The last chunk ends on the small first block so the
                # final output drain (and so the kernel tail) is short.
                tbs = tblocks
                if fc == n_chunks - 1 and len(tblocks) > 1:
                    tbs = tblocks[1:] + tblocks[:1]
                for (t0, nt) in tbs:
                    h = emit_stage_ph(fc, t0, nt)
                    if prev is not None:
                        emit_stage_w2(*prev, final=False)
                    prev = (fc, t0, nt, h)
            emit_stage_w2(*prev, final=True)

    nc.compile()
    _NC_CACHE[C] = nc
    return nc


def kernel(x, Wg, bg, W1, W2, W3, top_k):
    global LAST_RESULTS
    LAST_RESULTS = []
    x = np.ascontiguousarray(np.asarray(x), dtype=np.float32)
    Wg = np.asarray(Wg, dtype=np.float32)
    bg = np.asarray(bg, dtype=np.float32)
    W1 = np.asarray(W1, dtype=np.float32)
    W2 = np.asarray(W2, dtype=np.float32)
    W3 = np.asarray(W3, dtype=np.float32)
    k = int(top_k)
    B, S, D_ = x.shape
    T = B * S
    xt = x.reshape(T, D_)

    # Router (host): logits -> top-k -> softmax over the k selected.
    logits = xt @ Wg.T + bg
    order = np.argsort(-logits, axis=1, kind="stable")
    idx = order[:, :k]                              # [T, k]
    vals = np.take_along_axis(logits, idx, axis=1)
    ex = np.exp(vals - vals.max(axis=1, keepdims=True))
    wts = ex / ex.sum(axis=1, keepdims=True)        # [T, k]

    # Dispatch lists per expert.
    sel, wsel = [], []
    for e in range(E):
        mask = idx == e                             # [T, k]
        rows = np.nonzero(mask.any(axis=1))[0]
        sel.append(rows)
        wsel.append(wts[mask])                      # one weight per row
    max_ne = max(len(s) for s in sel)

    n_pass = max(1, math.ceil(max_ne / C_CAP))
    C = C_CAP if n_pass > 1 else max(256, 2 * math.ceil(max_ne / 2))
    nc = _build(C)

    # Pre-transposed per-expert weights in bf16.
    w1t = [np.ascontiguousarray(W1[e].T).astype(NP_BF16) for e in range(E)]
    w3t = [np.ascontiguousarray(W3[e].T).astype(NP_BF16) for e in range(E)]
    w2t = [np.ascontiguousarray(W2[e].T).astype(NP_BF16) for e in range(E)]
    xt_bf = xt.astype(NP_BF16)

    y = np.zeros((T, D_), dtype=np.float32)
    for p_i in range(n_pass):
        in_maps = []
        toks = []
        for e in range(E):
            tok = sel[e][p_i * C:(p_i + 1) * C]
            toks.append(tok)
            XT = np.zeros((D_, C), dtype=NP_BF16)
            if len(tok):
                XT[:, :len(tok)] = xt_bf[tok].T
            in_maps.append(
                {"xt": XT, "w1": w1t[e], "w3": w3t[e], "w2": w2t[e]}
            )
        res = bass_utils.run_bass_kernel_spmd(
            nc, in_maps, core_ids=list(range(NCORES))
        )
        LAST_RESULTS.append(res)
        for e in range(E):
            tok = toks[e]
            n = len(tok)
            if n == 0:
                continue
            out_e = res.results[e]["out"]           # [D, C]
            w_e = wsel[e][p_i * C:p_i * C + n]
            y[tok] += w_e[:, None] * out_e[:, :n].T

    return y.reshape(B, S, D_)


# revision 40
# speedup vs baseline: 1.0263x; 1.0263x over previous
"""MoE (top-2 routed SwiGLU) kernel for 8 Trainium2 NeuronCores.

Strategy (expert-parallel, host-routed dispatch):
  * Host: router matmul x@Wg.T (+bg), top-k + softmax weights, sort tokens
    by expert, pad each expert's token list to a shared capacity C.
  * Device (SPMD over 8 cores, core e owns expert e):
        OUT_e[D, C] = W2_e @ (silu(W1_e @ X_e) * (W3_e @ X_e))
    All matmul operands are bf16 (fp32 PSUM accumulate): bf16 gets the
    FWL fast weight-load path so LDWEIGHTS (~107ns) hides fully behind
    the N-column fill, and weight/input DMA bytes halve vs fp32.
    A burst of dummy matmuls at kernel start revs the HAM clock to
    full rate (2.4GHz) while the startup DMAs stream.
    Weights stream through SBUF once, chunked over the DFF axis; the
    output accumulates in SBUF across DFF chunks.
  * Host: y[tok] += w_tok_e * OUT_e[:, pos].T over the k experts per token.

Only top-k expert work is computed (4x less than the dense reference).
"""

import math
import sys

import numpy as np

for _p in ("/opt/trn_rl_repo", "/opt/pypackages"):
    if _p not in sys.path:
        sys.path.append(_p)

import ml_dtypes  # noqa: E402

import concourse.bass as bass  # noqa: E402
import concourse.tile as tile  # noqa: E402
from concourse import bacc, bass_utils, mybir  # noqa: E402

F32 = mybir.dt.float32
BF16 = mybir.dt.bfloat16
AF = mybir.ActivationFunctionType
NP_BF16 = ml_dtypes.bfloat16

D, DFF, E = 1024, 4096, 8
NCORES = 8
P = 128
KC = D // P            # 8 contraction chunks for the first matmuls
DFFC = 512             # dff columns per weight-stream chunk
NF = DFF // DFFC       # 8 weight-stream iterations
MC2 = DFFC // P        # 4 contraction chunks for the second matmul
C_CAP = 1280           # max padded tokens per expert per device pass
N_WARM = 136           # dummy matmuls: rev the HAM clock, then bridge
                       # the DMA wait so the PE never idles pre-stream

LAST_RESULTS = []      # BassKernelResults per device pass (for test harness)
_NC_CACHE = {}


def _install_ntff_hook():
    """Best-effort: register the axon NTFF profile hook so that
    BASS_TRACE=1 yields exec_time_ns even in a bare environment."""
    try:
        import types
        if "antenv.axon_hooks" not in sys.modules:
            mod = types.ModuleType("antenv.axon_hooks")
            holder = {}
            mod.set_axon_ntff_profile_hook = lambda h: holder.__setitem__("h", h)
            mod.get_axon_ntff_profile_hook = lambda: holder.get("h")
            sys.modules["antenv.axon_hooks"] = mod
            import antenv
            antenv.axon_hooks = mod
        mod = sys.modules["antenv.axon_hooks"]
        if mod.get_axon_ntff_profile_hook() is None:
            from trn_agent_boot.trn_boot import _ntff_profile_via_ctypes
            hook = _ntff_profile_via_ctypes("/opt/axon/libaxon_pjrt.so")
            if hook is not None:
                mod.set_axon_ntff_profile_hook(hook)
    except Exception:
        pass


_install_ntff_hook()


def _token_blocks(C):
    """Split C into even-sized blocks of <=512 columns (PSUM bank limit).
    A 256-col first block starts the PE on a small DMA prefix while
    keeping x rows >=512B (full DMA line rate); its ~3.4us of chunk-0
    work covers the second x block's wire time."""
    if C <= 512:
        return [(0, C)]
    first = 256
    rem = C - first
    n = max(1, math.ceil(rem / 500))
    base = ((rem // n) // 2) * 2
    extra = (rem - n * base) // 2         # blocks that get +2
    sizes = [first] + [base + 2] * extra + [base] * (n - extra)
    blocks, t0 = [], 0
    for sz in sizes:
        blocks.append((t0, sz))
        t0 += sz
    assert t0 == C and all(sz <= 512 for sz in sizes), (C, sizes)
    return blocks


def _build(C):
    """Compile the per-core expert-FFN program for capacity C."""
    if C in _NC_CACHE:
        return _NC_CACHE[C]
    nc = bacc.Bacc(
        "TRN2", target_bir_lowering=False, debug=False, num_devices=NCORES
    )
    x_d = nc.dram_tensor("xt", [D, C], BF16, kind="ExternalInput")
    w1_d = nc.dram_tensor("w1", [D, DFF], BF16, kind="ExternalInput")
    w3_d = nc.dram_tensor("w3", [D, DFF], BF16, kind="ExternalInput")
    w2_d = nc.dram_tensor("w2", [DFF, D], BF16, kind="ExternalInput")
    o_d = nc.dram_tensor("out", [D, C], F32, kind="ExternalOutput")

    xr = x_d.ap().rearrange("(kc p) c -> p kc c", p=P)
    w1r = w1_d.ap().rearrange("(kc p) f -> p kc f", p=P)
    w3r = w3_d.ap().rearrange("(kc p) f -> p kc f", p=P)
    w2r = w2_d.ap().rearrange("(kc p) d -> p kc d", p=P)
    orr = o_d.ap().rearrange("(mo p) c -> p mo c", p=P)

    tblocks = _token_blocks(C)
    # DFF chunk schedule: the first chunk is 256 wide so the startup
    # DMA prefix (first x block + 2x 0.5MB of weights, all >=512B rows
    # for full DMA line rate) is small and the PE starts early; steady
    # state streams 512-wide chunks.
    chunks = [(0, 256), (256, 256)]
    off = 512
    while off < DFF:
        chunks.append((off, 512))
        off += 512
    n_chunks = len(chunks)

    with tile.TileContext(nc) as tc:
        with (
            tc.tile_pool(name="res", bufs=1) as res,
            tc.tile_pool(name="w13", bufs=2) as w13,
            tc.tile_pool(name="w2p", bufs=3) as w2p,
            tc.tile_pool(name="hp", bufs=3) as hp,
            tc.tile_pool(name="sp", bufs=3) as sp,
            tc.tile_pool(name="ps13", bufs=2, space="PSUM") as ps13,
            tc.tile_pool(name="pso", bufs=3, space="PSUM") as pso,
            tc.tile_pool(name="psw", bufs=1, space="PSUM") as psw,
        ):
            xt = res.tile([P, KC, C], BF16, tag="xt")
            acc = res.tile([P, KC, C], F32, tag="acc")

            # HAM pre-warm: a burst of tiny matmuls revs the PE clock to
            # full rate while the startup DMAs stream in parallel.
            dwm = res.tile([P, P], BF16, tag="dwm")
            nc.vector.memset(dwm[:, :], 0.0)
            pd = psw.tile([P, P], F32, tag="pd")
            for _ in range(N_WARM):
                nc.tensor.matmul(pd[:, :], dwm[:, :], dwm[:, :],
                                 start=True, stop=True)

            # dma_start costs 0.6-3us of SERIAL issue time on the
            # issuing engine (scaling with descriptor rows), issue from
            # multiple engines contends at the shared DGE, and the ring
            # interleaves packets of queued transfers (so a transfer
            # completes only after ~everything issued before it). Hence:
            # whole-tile DMAs, one issuing engine, strict need-order.
            def load_x(t0, nt):
                nc.sync.dma_start(
                    xt[:, :, t0:t0 + nt], xr[:, :, t0:t0 + nt]
                )

            def load_w13(fs, dffc):
                w1t = w13.tile([P, KC, DFFC], BF16, tag="w1")
                w3t = w13.tile([P, KC, DFFC], BF16, tag="w3")
                nc.sync.dma_start(w1t[:, :, :dffc], w1r[:, :, fs:fs + dffc])
                nc.sync.dma_start(w3t[:, :, :dffc], w3r[:, :, fs:fs + dffc])
                return w1t, w3t

            def load_w2(fs, dffc):
                w2t = w2p.tile([P, MC2, D], BF16, tag="w2")
                mc2 = dffc // P
                nc.sync.dma_start(
                    w2t[:, :mc2, :], w2r[:, fs // P:fs // P + mc2, :]
                )
                return w2t

            # Startup in need-order: the first token block of x, chunk-0
            # W1/W3, the second x block, chunk-0 W2, rest of x.
            load_x(*tblocks[0])
            w13_of = {0: load_w13(0, chunks[0][1])}
            if len(tblocks) > 1:
                load_x(*tblocks[1])
            w2_of = {0: load_w2(0, chunks[0][1])}
            for tb in tblocks[2:]:
                load_x(*tb)

            def emit_stage_ph(fc, t0, nt):
                """First matmul pair + silu*mul for one (chunk, tblock)."""
                dffc = chunks[fc][1]
                h = hp.tile([P, MC2, 512], BF16, tag="h")
                for m in range(dffc // P):
                    (w1s, w3s), cl = w13_of[fc], m * P
                    ph1 = ps13.tile([P, 512], F32, tag="ph1")
                    ph3 = ps13.tile([P, 512], F32, tag="ph3")
                    for k in range(KC):
                        nc.tensor.matmul(
                            ph1[:, :nt],
                            w1s[:, k, cl:cl + P],
                            xt[:, k, t0:t0 + nt],
                            start=(k == 0),
                            stop=(k == KC - 1),
                        )
                    for k in range(KC):
                        nc.tensor.matmul(
                            ph3[:, :nt],
                            w3s[:, k, cl:cl + P],
                            xt[:, k, t0:t0 + nt],
                            start=(k == 0),
                            stop=(k == KC - 1),
                        )
                    s = sp.tile([P, 512], BF16, tag="s")
                    nc.scalar.activation(s[:, :nt], ph1[:, :nt], AF.Silu)
                    nc.vector.tensor_mul(h[:, m, :nt], s[:, :nt], ph3[:, :nt])
                return h

            def emit_stage_w2(fc, t0, nt, h, final):
                """Second matmul + accumulate for one (chunk, tblock)."""
                fs, dffc = chunks[fc]
                mc2 = dffc // P
                w2t = w2_of[fc]
                for mo in range(KC):
                    po = pso.tile([P, 512], F32, tag="po")
                    for j in range(mc2):
                        nc.tensor.matmul(
                            po[:, :nt],
                            w2t[:, j, mo * P:(mo + 1) * P],
                            h[:, j, :nt],
                            start=(j == 0),
                            stop=(j == mc2 - 1),
                        )
                    if fc == 0:
                        nc.scalar.activation(
                            acc[:, mo, t0:t0 + nt], po[:, :nt], AF.Copy
                        )
                    else:
                        nc.vector.tensor_add(
                            acc[:, mo, t0:t0 + nt],
                            acc[:, mo, t0:t0 + nt],
                            po[:, :nt],
                        )
                    # Stream the finished output out while later stages
                    # still compute. The final stage is the small token
                    # block, drained in two halves so the second half's
                    # issue+transfer is all that trails the last matmul.
                    if fc == n_chunks - 1:
                        if not final:
                            nc.sync.dma_start(
                                orr[:, mo, t0:t0 + nt],
                                acc[:, mo, t0:t0 + nt],
                            )
                        elif mo in (KC // 2 - 1, KC - 1):
                            lo = 0 if mo == KC // 2 - 1 else KC // 2
                            nc.sync.dma_start(
                                orr[:, lo:mo + 1, t0:t0 + nt],
                                acc[:, lo:mo + 1, t0:t0 + nt],
                            )

            # Software pipeline: the W2 stage lags one (chunk, tblock)
            # stage behind the ph stage, so the PE never waits on the
            # scalar/vector silu*mul chain at stage boundaries.
            prev = None
            for fc in range(n_chunks):
                if fc >= 1:
                    fs, dffc = chunks[fc]
                    w13_of[fc] = load_w13(fs, dffc)
                    w2_of[fc] = load_w2(fs, dffc)
                # The last chunk ends on the small first block so the
                # final output drain (and so the kernel tail) is short.
                tbs = tblocks
                if fc == n_chunks - 1 and len(tblocks) > 1:
                    tbs = tblocks[1:] + tblocks[:1]
                for (t0, nt) in tbs:
                    h = emit_stage_ph(fc, t0, nt)
                    if prev is not None:
                        emit_stage_w2(*prev, final=False)
                    prev = (fc, t0, nt, h)
            emit_stage_w2(*prev, final=True)

    nc.compile()
    _NC_CACHE[C] = nc
    return nc


def kernel(x, Wg, bg, W1, W2, W3, top_k):
    global LAST_RESULTS
    LAST_RESULTS = []
    x = np.ascontiguousarray(np.asarray(x), dtype=np.float32)
    Wg = np.asarray(Wg, dtype=np.float32)
    bg = np.asarray(bg, dtype=np.float32)
    W1 = np.asarray(W1, dtype=np.float32)
    W2 = np.asarray(W2, dtype=np.float32)
    W3 = np.asarray(W3, dtype=np.float32)
    k = int(top_k)
    B, S, D_ = x.shape
    T = B * S
    xt = x.reshape(T, D_)

    # Router (host): logits -> top-k -> softmax over the k selected.
    logits = xt @ Wg.T + bg
    order = np.argsort(-logits, axis=1, kind="stable")
    idx = order[:, :k]                              # [T, k]
    vals = np.take_along_axis(logits, idx, axis=1)
    ex = np.exp(vals - vals.max(axis=1, keepdims=True))
    wts = ex / ex.sum(axis=1, keepdims=True)        # [T, k]

    # Dispatch lists per expert.
    sel, wsel = [], []
    for e in range(E):
        mask = idx == e                             # [T, k]
        rows = np.nonzero(mask.any(axis=1))[0]
        sel.append(rows)
        wsel.append(wts[mask])                      # one weight per row
    max_ne = max(len(s) for s in sel)

    n_pass = max(1, math.ceil(max_ne / C_CAP))
    C = C_CAP if n_pass > 1 else max(256, 2 * math.ceil(max_ne / 2))
    nc = _build(C)

    # Pre-transposed per-expert weights in bf16.
    w1t = [np.ascontiguousarray(W1[e].T).astype(NP_BF16) for e in range(E)]
    w3t = [np.ascontiguousarray(W3[e].T).astype(NP_BF16) for e in range(E)]
    w2t = [np.ascontiguousarray(W2[e].T).astype(NP_BF16) for e in range(E)]
    xt_bf = xt.astype(NP_BF16)

    y = np.zeros((T, D_), dtype=np.float32)
    for p_i in range(n_pass):
        in_maps = []
        toks = []
        for e in range(E):
            tok = sel[e][p_i * C:(p_i + 1) * C]
            toks.append(tok)
            XT = np.zeros((D_, C), dtype=NP_BF16)
            if len(tok):
                XT[:, :len(tok)] = xt_bf[tok].T
            in_maps.append(
                {"xt": XT, "w1": w1t[e], "w3": w3t[e], "w2": w2t[e]}
            )
        res = bass_utils.run_bass_kernel_spmd(
            nc, in_maps, core_ids=list(range(NCORES))
        )
        LAST_RESULTS.append(res)
        for e in range(E):
            tok = toks[e]
            n = len(tok)
            if n == 0:
                continue
            out_e = res.results[e]["out"]           # [D, C]
            w_e = wsel[e][p_i * C:p_i * C + n]
            y[tok] += w_e[:, None] * out_e[:, :n].T

    return y.reshape(B, S, D_)


# revision 41
# speedup vs baseline: 1.0319x; 1.0055x over previous
"""MoE (top-2 routed SwiGLU) kernel for 8 Trainium2 NeuronCores.

Strategy (expert-parallel, host-routed dispatch):
  * Host: router matmul x@Wg.T (+bg), top-k + softmax weights, sort tokens
    by expert, pad each expert's token list to a shared capacity C.
  * Device (SPMD over 8 cores, core e owns expert e):
        OUT_e[D, C] = W2_e @ (silu(W1_e @ X_e) * (W3_e @ X_e))
    All matmul operands are bf16 (fp32 PSUM accumulate): bf16 gets the
    FWL fast weight-load path so LDWEIGHTS (~107ns) hides fully behind
    the N-column fill, and weight/input DMA bytes halve vs fp32.
    A burst of dummy matmuls at kernel start revs the HAM clock to
    full rate (2.4GHz) while the startup DMAs stream.
    Weights stream through SBUF once, chunked over the DFF axis; the
    output accumulates in SBUF across DFF chunks.
  * Host: y[tok] += w_tok_e * OUT_e[:, pos].T over the k experts per token.

Only top-k expert work is computed (4x less than the dense reference).
"""

import math
import sys

import numpy as np

for _p in ("/opt/trn_rl_repo", "/opt/pypackages"):
    if _p not in sys.path:
        sys.path.append(_p)

import ml_dtypes  # noqa: E402

import concourse.bass as bass  # noqa: E402
import concourse.tile as tile  # noqa: E402
from concourse import bacc, bass_utils, mybir  # noqa: E402

F32 = mybir.dt.float32
BF16 = mybir.dt.bfloat16
AF = mybir.ActivationFunctionType
NP_BF16 = ml_dtypes.bfloat16

D, DFF, E = 1024, 4096, 8
NCORES = 8
P = 128
KC = D // P            # 8 contraction chunks for the first matmuls
DFFC = 512             # dff columns per weight-stream chunk
NF = DFF // DFFC       # 8 weight-stream iterations
MC2 = DFFC // P        # 4 contraction chunks for the second matmul
C_CAP = 1280           # max padded tokens per expert per device pass
N_WARM = 100           # dummy matmuls: rev the HAM clock, then bridge
                       # the DMA wait so the PE never idles pre-stream

LAST_RESULTS = []      # BassKernelResults per device pass (for test harness)
_NC_CACHE = {}


def _install_ntff_hook():
    """Best-effort: register the axon NTFF profile hook so that
    BASS_TRACE=1 yields exec_time_ns even in a bare environment."""
    try:
        import types
        if "antenv.axon_hooks" not in sys.modules:
            mod = types.ModuleType("antenv.axon_hooks")
            holder = {}
            mod.set_axon_ntff_profile_hook = lambda h: holder.__setitem__("h", h)
            mod.get_axon_ntff_profile_hook = lambda: holder.get("h")
            sys.modules["antenv.axon_hooks"] = mod
            import antenv
            antenv.axon_hooks = mod
        mod = sys.modules["antenv.axon_hooks"]
        if mod.get_axon_ntff_profile_hook() is None:
            from trn_agent_boot.trn_boot import _ntff_profile_via_ctypes
            hook = _ntff_profile_via_ctypes("/opt/axon/libaxon_pjrt.so")
            if hook is not None:
                mod.set_axon_ntff_profile_hook(hook)
    except Exception:
        pass


_install_ntff_hook()


def _token_blocks(C):
    """Split C into even-sized blocks of <=512 columns (PSUM bank limit).
    A 256-col first block starts the PE on a small DMA prefix while
    keeping x rows >=512B (full DMA line rate); its ~3.4us of chunk-0
    work covers the second x block's wire time."""
    if C <= 512:
        return [(0, C)]
    first = 256
    rem = C - first
    n = max(1, math.ceil(rem / 500))
    base = ((rem // n) // 2) * 2
    extra = (rem - n * base) // 2         # blocks that get +2
    sizes = [first] + [base + 2] * extra + [base] * (n - extra)
    blocks, t0 = [], 0
    for sz in sizes:
        blocks.append((t0, sz))
        t0 += sz
    assert t0 == C and all(sz <= 512 for sz in sizes), (C, sizes)
    return blocks


def _build(C):
    """Compile the per-core expert-FFN program for capacity C."""
    if C in _NC_CACHE:
        return _NC_CACHE[C]
    nc = bacc.Bacc(
        "TRN2", target_bir_lowering=False, debug=False, num_devices=NCORES
    )
    x_d = nc.dram_tensor("xt", [D, C], BF16, kind="ExternalInput")
    w1_d = nc.dram_tensor("w1", [D, DFF], BF16, kind="ExternalInput")
    w3_d = nc.dram_tensor("w3", [D, DFF], BF16, kind="ExternalInput")
    w2_d = nc.dram_tensor("w2", [DFF, D], BF16, kind="ExternalInput")
    o_d = nc.dram_tensor("out", [D, C], F32, kind="ExternalOutput")

    xr = x_d.ap().rearrange("(kc p) c -> p kc c", p=P)
    w1r = w1_d.ap().rearrange("(kc p) f -> p kc f", p=P)
    w3r = w3_d.ap().rearrange("(kc p) f -> p kc f", p=P)
    w2r = w2_d.ap().rearrange("(kc p) d -> p kc d", p=P)
    orr = o_d.ap().rearrange("(mo p) c -> p mo c", p=P)

    tblocks = _token_blocks(C)
    # DFF chunk schedule: the first chunk is 256 wide so the startup
    # DMA prefix (first x block + 2x 0.5MB of weights, all >=512B rows
    # for full DMA line rate) is small and the PE starts early; steady
    # state streams 512-wide chunks.
    chunks = [(0, 256), (256, 256)]
    off = 512
    while off < DFF:
        chunks.append((off, 512))
        off += 512
    n_chunks = len(chunks)

    with tile.TileContext(nc) as tc:
        with (
            tc.tile_pool(name="res", bufs=1) as res,
            tc.tile_pool(name="w13", bufs=2) as w13,
            tc.tile_pool(name="w2p", bufs=3) as w2p,
            tc.tile_pool(name="hp", bufs=3) as hp,
            tc.tile_pool(name="sp", bufs=3) as sp,
            tc.tile_pool(name="ps13", bufs=2, space="PSUM") as ps13,
            tc.tile_pool(name="pso", bufs=3, space="PSUM") as pso,
            tc.tile_pool(name="psw", bufs=1, space="PSUM") as psw,
        ):
            xt = res.tile([P, KC, C], BF16, tag="xt")
            acc = res.tile([P, KC, C], F32, tag="acc")

            # HAM pre-warm: a burst of tiny matmuls revs the PE clock to
            # full rate while the startup DMAs stream in parallel.
            dwm = res.tile([P, P], BF16, tag="dwm")
            nc.vector.memset(dwm[:, :], 0.0)
            pd = psw.tile([P, P], F32, tag="pd")
            for _ in range(N_WARM):
                nc.tensor.matmul(pd[:, :], dwm[:, :], dwm[:, :],
                                 start=True, stop=True)

            # dma_start costs 0.6-3us of SERIAL issue time on the
            # issuing engine (scaling with descriptor rows), issue from
            # multiple engines contends at the shared DGE, and the ring
            # interleaves packets of queued transfers (so a transfer
            # completes only after ~everything issued before it). Hence:
            # whole-tile DMAs, one issuing engine, strict need-order.
            def load_x(t0, nt):
                nc.sync.dma_start(
                    xt[:, :, t0:t0 + nt], xr[:, :, t0:t0 + nt]
                )

            def load_w13(fs, dffc):
                w1t = w13.tile([P, KC, DFFC], BF16, tag="w1")
                w3t = w13.tile([P, KC, DFFC], BF16, tag="w3")
                nc.sync.dma_start(w1t[:, :, :dffc], w1r[:, :, fs:fs + dffc])
                nc.sync.dma_start(w3t[:, :, :dffc], w3r[:, :, fs:fs + dffc])
                return w1t, w3t

            def load_w2(fs, dffc):
                w2t = w2p.tile([P, MC2, D], BF16, tag="w2")
                mc2 = dffc // P
                nc.sync.dma_start(
                    w2t[:, :mc2, :], w2r[:, fs // P:fs // P + mc2, :]
                )
                return w2t

            # Startup in need-order: the first token block of x, chunk-0
            # W1/W3, the second x block, chunk-0 W2, rest of x.
            load_x(*tblocks[0])
            w13_of = {0: load_w13(0, chunks[0][1])}
            if len(tblocks) > 1:
                load_x(*tblocks[1])
            w2_of = {0: load_w2(0, chunks[0][1])}
            for tb in tblocks[2:]:
                load_x(*tb)

            def emit_stage_ph(fc, t0, nt):
                """First matmul pair + silu*mul for one (chunk, tblock)."""
                dffc = chunks[fc][1]
                h = hp.tile([P, MC2, 512], BF16, tag="h")
                for m in range(dffc // P):
                    (w1s, w3s), cl = w13_of[fc], m * P
                    ph1 = ps13.tile([P, 512], F32, tag="ph1")
                    ph3 = ps13.tile([P, 512], F32, tag="ph3")
                    for k in range(KC):
                        nc.tensor.matmul(
                            ph1[:, :nt],
                            w1s[:, k, cl:cl + P],
                            xt[:, k, t0:t0 + nt],
                            start=(k == 0),
                            stop=(k == KC - 1),
                        )
                    for k in range(KC):
                        nc.tensor.matmul(
                            ph3[:, :nt],
                            w3s[:, k, cl:cl + P],
                            xt[:, k, t0:t0 + nt],
                            start=(k == 0),
                            stop=(k == KC - 1),
                        )
                    s = sp.tile([P, 512], BF16, tag="s")
                    nc.scalar.activation(s[:, :nt], ph1[:, :nt], AF.Silu)
                    nc.vector.tensor_mul(h[:, m, :nt], s[:, :nt], ph3[:, :nt])
                return h

            def emit_stage_w2(fc, t0, nt, h, final):
                """Second matmul + accumulate for one (chunk, tblock)."""
                fs, dffc = chunks[fc]
                mc2 = dffc // P
                w2t = w2_of[fc]
                for mo in range(KC):
                    po = pso.tile([P, 512], F32, tag="po")
                    for j in range(mc2):
                        nc.tensor.matmul(
                            po[:, :nt],
                            w2t[:, j, mo * P:(mo + 1) * P],
                            h[:, j, :nt],
                            start=(j == 0),
                            stop=(j == mc2 - 1),
                        )
                    if fc == 0:
                        nc.scalar.activation(
                            acc[:, mo, t0:t0 + nt], po[:, :nt], AF.Copy
                        )
                    else:
                        nc.vector.tensor_add(
                            acc[:, mo, t0:t0 + nt],
                            acc[:, mo, t0:t0 + nt],
                            po[:, :nt],
                        )
                    # Stream the finished output out while later stages
                    # still compute. The final stage is the small token
                    # block, drained in two halves so the second half's
                    # issue+transfer is all that trails the last matmul.
                    if fc == n_chunks - 1:
                        if not final:
                            nc.sync.dma_start(
                                orr[:, mo, t0:t0 + nt],
                                acc[:, mo, t0:t0 + nt],
                            )
                        elif mo in (KC // 2 - 1, KC - 1):
                            lo = 0 if mo == KC // 2 - 1 else KC // 2
                            nc.sync.dma_start(
                                orr[:, lo:mo + 1, t0:t0 + nt],
                                acc[:, lo:mo + 1, t0:t0 + nt],
                            )

            # Software pipeline: the W2 stage lags one (chunk, tblock)
            # stage behind the ph stage, so the PE never waits on the
            # scalar/vector silu*mul chain at stage boundaries.
            prev = None
            for fc in range(n_chunks):
                if fc >= 1:
                    fs, dffc = chunks[fc]
                    w13_of[fc] = load_w13(fs, dffc)
                    w2_of[fc] = load_w2(fs, dffc)
                # The last chunk ends on the small first block so the
                # final output drain (and so the kernel tail) is short.
                tbs = tblocks
                if fc == n_chunks - 1 and len(tblocks) > 1:
                    tbs = tblocks[1:] + tblocks[:1]
                for (t0, nt) in tbs:
                    h = emit_stage_ph(fc, t0, nt)
                    if prev is not None:
                        emit_stage_w2(*prev, final=False)
                    prev = (fc, t0, nt, h)
            emit_stage_w2(*prev, final=True)

    nc.compile()
    _NC_CACHE[C] = nc
    return nc


def kernel(x, Wg, bg, W1, W2, W3, top_k):
    global LAST_RESULTS
    LAST_RESULTS = []
    x = np.ascontiguousarray(np.asarray(x), dtype=np.float32)
    Wg = np.asarray(Wg, dtype=np.float32)
    bg = np.asarray(bg, dtype=np.float32)
    W1 = np.asarray(W1, dtype=np.float32)
    W2 = np.asarray(W2, dtype=np.float32)
    W3 = np.asarray(W3, dtype=np.float32)
    k = int(top_k)
    B, S, D_ = x.shape
    T = B * S
    xt = x.reshape(T, D_)

    # Router (host): logits -> top-k -> softmax over the k selected.
    logits = xt @ Wg.T + bg
    order = np.argsort(-logits, axis=1, kind="stable")
    idx = order[:, :k]                              # [T, k]
    vals = np.take_along_axis(logits, idx, axis=1)
    ex = np.exp(vals - vals.max(axis=1, keepdims=True))
    wts = ex / ex.sum(axis=1, keepdims=True)        # [T, k]

    # Dispatch lists per expert.
    sel, wsel = [], []
    for e in range(E):
        mask = idx == e                             # [T, k]
        rows = np.nonzero(mask.any(axis=1))[0]
        sel.append(rows)
        wsel.append(wts[mask])                      # one weight per row
    max_ne = max(len(s) for s in sel)

    n_pass = max(1, math.ceil(max_ne / C_CAP))
    C = C_CAP if n_pass > 1 else max(256, 2 * math.ceil(max_ne / 2))
    nc = _build(C)

    # Pre-transposed per-expert weights in bf16.
    w1t = [np.ascontiguousarray(W1[e].T).astype(NP_BF16) for e in range(E)]
    w3t = [np.ascontiguousarray(W3[e].T).astype(NP_BF16) for e in range(E)]
    w2t = [np.ascontiguousarray(W2[e].T).astype(NP_BF16) for e in range(E)]
    xt_bf = xt.astype(NP_BF16)

    y = np.zeros((T, D_), dtype=np.float32)
    for p_i in range(n_pass):
        in_maps = []
        toks = []
        for e in range(E):
            tok = sel[e][p_i * C:(p_i + 1) * C]
            toks.append(tok)
            XT = np.zeros((D_, C), dtype=NP_BF16)
            if len(tok):
                XT[:, :len(tok)] = xt_bf[tok].T
            in_maps.append(
                {"xt": XT, "w1": w1t[e], "w3": w3t[e], "w2": w2t[e]}
            )
        res = bass_utils.run_bass_kernel_spmd(
            nc, in_maps, core_ids=list(range(NCORES))
        )
        LAST_RESULTS.append(res)
        for e in range(E):
            tok = toks[e]
            n = len(tok)
            if n == 0:
                continue
            out_e = res.results[e]["out"]           # [D, C]
            w_e = wsel[e][p_i * C:p_i * C + n]
            y[tok] += w_e[:, None] * out_e[:, :n].T

    return y.reshape(B, S, D_)
